# revision 1
# baseline (speedup 1.0000x reference)
"""Multi-head attention forward (B=8, N=1024, C=768, H=12) on 8 TRN2 NeuronCores.

Sharding: data-parallel over batch — core b computes batch b end-to-end
(weights replicated, no collectives). Per-core dataflow, all matmuls bf16
with fp32 PSUM accumulation:

  x [1024,768] --cast+PE transpose (batched per seq block)--> xT [768,1024]
  qT,kT[t] = w_qkv-stationary matmuls over xT               [feat, seq]
  v        = xT-stationary matmuls over w_qkv[:, 1536:]     [seq, feat(+ones)]
  S^T  = kT-stationary matmuls over qT (2 heads packed in PE row groups)
  E^T  = exp(S^T / 8) via ACT straight from PSUM (no max-sub needed)
  PV   = v_aug-stationary matmuls over E^T -> [out^T ; rowsum] in PSUM
  out^T = PV[0:64] * (1/rowsum)   (gpsimd partition-broadcast + fast recip)
  y = out^T-stationary matmuls over w_proj + bias

Scheduling keeps PE and ACT co-busy (Tile executes per-engine streams in
program order): v-computation is woven into the first head-pair's attention
between exp and PV (PV(jp) only needs vn[0..2jp+1]); the next head-pair's
q/k matmuls are woven one psum-group per jp into the previous pair's second
i-half so the exp queue never drains at head boundaries; w_proj loads are
deferred past the prologue; x and the weights are pre-cast to bf16 on the
host so input DMA is half-width and lands directly in persistent tiles.
Measured ~199-202 us per core, output max rel err 8.0e-3 vs the fp32
reference (bf16 storage of q/k/E/v dominates).
"""
import numpy as np
from contextlib import ExitStack

import concourse.bacc as bacc
import concourse.tile as tile
from concourse import mybir, bass_utils, masks
from concourse.tile import add_dep_helper

F32 = mybir.dt.float32
F32R = mybir.dt.float32r
BF16 = mybir.dt.bfloat16
EXP = mybir.ActivationFunctionType.Exp

# matmul operand dtypes per stage
QKV_DT = BF16   # x^T, w_qkv  (feeds q,k -> scores; keep precise)
ATT_DT = BF16   # q^T, k^T, v, E^T  (S and PV matmuls)
PROJ_DT = BF16  # out^T, w_proj

B = 8
N = 1024       # sequence length
C = 768        # channels
H = 12         # heads
HD = 64        # head dim
NB = N // 128  # 8 seq blocks
CB = C // 128  # 6 channel chunks
HP = H // 2    # 6 head pairs
VW = HD + 1    # 65: v columns per head incl. ones column
SCALE = float(HD) ** -0.5

_NC = None


def _build():
    nc = bacc.Bacc("TRN2", target_bir_lowering=False, debug=False, num_devices=B)
    x = nc.dram_tensor("x", [N, C], BF16, kind="ExternalInput")
    w_qkv = nc.dram_tensor("w_qkv", [C, 3 * C], BF16, kind="ExternalInput")
    w_proj = nc.dram_tensor("w_proj", [C, C], BF16, kind="ExternalInput")
    b_proj = nc.dram_tensor("b_proj", [1, C], F32, kind="ExternalInput")
    y = nc.dram_tensor("y", [N, C], F32, kind="ExternalOutput")

    with tile.TileContext(nc) as tc, ExitStack() as ctx:
        const = ctx.enter_context(tc.tile_pool(name="const", bufs=1))
        p_qk = ctx.enter_context(tc.tile_pool(name="p_qk", bufs=1))
        p_v = ctx.enter_context(tc.tile_pool(name="p_v", bufs=1))
        p_out = ctx.enter_context(tc.tile_pool(name="p_out", bufs=1))
        p_wp = ctx.enter_context(tc.tile_pool(name="p_wp", bufs=1))

        ident_bf = const.tile([128, 128], BF16, tag="ident_bf")
        masks.make_identity(nc, ident_bf[:])
        bias_row = const.tile([1, C], F32, tag="bias_row")
        nc.sync.dma_start(bias_row[:], b_proj.ap())
        bias_bc = const.tile([128, C], F32, tag="bias_bc")
        nc.gpsimd.partition_broadcast(bias_bc[:], bias_row[:])
        ones12 = const.tile([128, H], F32, tag="ones12")
        nc.vector.memset(ones12[:], 1.0)

        qT = [p_qk.tile([128, N], ATT_DT, tag=f"qT{t}", name=f"qT{t}") for t in range(HP)]
        kT = [p_qk.tile([128, N], ATT_DT, tag=f"kT{t}", name=f"kT{t}") for t in range(HP)]
        vn = [p_v.tile([128, H * VW], ATT_DT, tag=f"v{ib}", name=f"v{ib}") for ib in range(NB)]
        outT = [p_out.tile([128, N], PROJ_DT, tag=f"outT{t}", name=f"outT{t}") for t in range(HP)]
        wp = [p_wp.tile([128, C], PROJ_DT, tag=f"wp{t}", name=f"wp{t}") for t in range(CB)]

        with (
            tc.tile_pool(name="p_xT", bufs=1) as p_xT,
            tc.tile_pool(name="p_xin", bufs=4) as p_xin,
            tc.tile_pool(name="p_wq", bufs=1) as p_wq,
            tc.tile_pool(name="p_E", bufs=7) as p_E,
            tc.tile_pool(name="p_nrm", bufs=2) as p_nrm,
            tc.tile_pool(name="p_y", bufs=2) as p_y,
            tc.tile_pool(name="ps_mm", bufs=2, space="PSUM") as ps_mm,
            tc.tile_pool(name="ps_s", bufs=2, space="PSUM") as ps_s,
            tc.tile_pool(name="ps_pv", bufs=1, space="PSUM") as ps_pv,
        ):
            # ---- x^T via PE transposes (starts immediately; weight DMA
            # proceeds in parallel on other queues)
            xT = [p_xT.tile([128, N], QKV_DT, tag=f"xT{c}", name=f"xT{c}")
                  for c in range(CB)]
            for ib in range(NB):
                xb = p_xin.tile([128, C], BF16, tag="xb")
                nc.sync.dma_start(xb[:], x.ap()[ib * 128:(ib + 1) * 128, :])
                pt = ps_mm.tile([128, C], BF16, tag="pmm", name=f"ptr{ib}")
                for cc in range(CB):
                    nc.tensor.transpose(pt[:, cc * 128:(cc + 1) * 128],
                                        xb[:, cc * 128:(cc + 1) * 128], ident_bf[:])
                for cc in range(CB):
                    dst = xT[cc][:, ib * 128:(ib + 1) * 128]
                    src = pt[:, cc * 128:(cc + 1) * 128]
                    if cc % 2 == 0:
                        nc.scalar.copy(dst, src)
                    else:
                        nc.vector.tensor_copy(dst, src)

            wq = []
            for cc in range(CB):
                wt = p_wq.tile([128, 3 * C], QKV_DT, tag=f"wq{cc}", name=f"wq{cc}")
                nc.sync.dma_start(wt[:, 0:2 * C], w_qkv.ap()[cc * 128:(cc + 1) * 128, 0:2 * C])
                wq.append(wt)
            for cc in range(CB):
                nc.sync.dma_start(wq[cc][:, 2 * C:3 * C],
                                  w_qkv.ap()[cc * 128:(cc + 1) * 128, 2 * C:3 * C])

            def emit_qk_group(t, gi):
                f_off, dst = ((0, qT), (C, kT))[gi // 2]
                nh = gi % 2
                pq = ps_mm.tile([128, 512], F32, tag="pmm", name=f"pq{t}_{gi}")
                for cc in range(CB):
                    nc.tensor.matmul(
                        pq[:],
                        wq[cc][:, f_off + t * 128: f_off + (t + 1) * 128],
                        xT[cc][:, nh * 512:(nh + 1) * 512],
                        start=(cc == 0), stop=(cc == CB - 1))
                nc.vector.tensor_copy(dst[t][:, nh * 512:(nh + 1) * 512], pq[:])

            def emit_qk(t):
                for gi in range(4):
                    emit_qk_group(t, gi)

            def emit_v_group(ib, half):
                pv = ps_mm.tile([128, 384], F32, tag="pmm", name=f"pv{ib}_{half}")
                for cc in range(CB):
                    nc.tensor.matmul(
                        pv[:],
                        xT[cc][:, ib * 128:(ib + 1) * 128],
                        wq[cc][:, 2 * C + half * 384: 2 * C + (half + 1) * 384],
                        start=(cc == 0), stop=(cc == CB - 1))
                nc.vector.tensor_copy(
                    vn[ib][:, half * 6 * VW:(half + 1) * 6 * VW]
                    .rearrange("p (h d) -> p h d", d=VW)[:, :, 0:HD],
                    pv[:].rearrange("p (h d) -> p h d", d=HD))
                if half == 1:
                    nc.vector.tensor_copy(
                        vn[ib][:].rearrange("p (h d) -> p h d", d=VW)[:, :, HD:VW],
                        ones12[:])

            emit_qk(0)
            for ib0 in range(2):
                for h0 in range(2):
                    emit_v_group(ib0, h0)

            # ---- per head-pair: q^T,k^T then attention (pipelines across t)
            for t in range(HP):
                hA, hB = 2 * t, 2 * t + 1
                for ih in range(2):
                    pre_pv = None
                    if t == 0 and ih == 0:
                        pre_pv = [
                            [lambda ib=ib, h=h: emit_v_group(ib, h)
                             for ib in (2 * jp_ + 2, 2 * jp_ + 3) for h in (0, 1)]
                            for jp_ in range(3)
                        ]
                    elif ih == 1 and t + 1 < HP:
                        # weave next head-pair's q/k matmuls one psum-group per
                        # jp so the ACT exp queue never drains at the boundary
                        pre_pv = [[lambda g=g, tt=t: emit_qk_group(tt + 1, g)]
                                  for g in range(4)]
                    ppA = ps_pv.tile([VW, 512], F32, tag="pvA", name=f"ppA{t}{ih}")
                    ppB = ps_pv.tile([VW, 512], F32, tag="pvB", name=f"ppB{t}{ih}")
                    for jp in range(4):
                        jbs = (2 * jp, 2 * jp + 1)
                        sA = ps_s.tile([128, 1024], F32, tag="s2", name=f"sA{t}{ih}{jp}")
                        sB = ps_s.tile([128, 1024], F32, tag="s2", name=f"sB{t}{ih}{jp}")
                        for jb, co in zip(jbs, (0, 512)):
                            nc.tensor.matmul(
                                sA[:, co:co + 512],
                                kT[t][0:64, jb * 128:(jb + 1) * 128],
                                qT[t][0:64, ih * 512:(ih + 1) * 512],
                                start=True, stop=True, tile_position=(0, 0))
                            nc.tensor.matmul(
                                sB[:, co:co + 512],
                                kT[t][64:128, jb * 128:(jb + 1) * 128],
                                qT[t][64:128, ih * 512:(ih + 1) * 512],
                                start=True, stop=True, tile_position=(64, 0))
                        eA = p_E.tile([128, 1024], ATT_DT, tag="e2", name=f"eA{t}{ih}{jp}")
                        eB = p_E.tile([128, 1024], ATT_DT, tag="e2", name=f"eB{t}{ih}{jp}")
                        nc.scalar.activation(eA[:], sA[:], EXP, scale=SCALE)
                        nc.scalar.activation(eB[:], sB[:], EXP, scale=SCALE)
                        if pre_pv is not None and jp < len(pre_pv):
                            for thunk in pre_pv[jp]:
                                thunk()
                        for jb, co in zip(jbs, (0, 512)):
                            nc.tensor.matmul(
                                ppA[:], vn[jb][:, hA * VW:(hA + 1) * VW],
                                eA[:, co:co + 512],
                                start=(jb == 0), stop=(jb == NB - 1))
                            nc.tensor.matmul(
                                ppB[:], vn[jb][:, hB * VW:(hB + 1) * VW],
                                eB[:, co:co + 512],
                                start=(jb == 0), stop=(jb == NB - 1))
                    # normalize: out^T = PV[0:64] / rowsum
                    for pp, po in ((ppA, 0), (ppB, 64)):
                        rs = p_nrm.tile([1, 512], F32, tag="rs", name=f"rs{t}{ih}{po}")
                        nc.vector.tensor_copy(rs[:], pp[HD:VW, :])
                        bc = p_nrm.tile([64, 512], F32, tag="bc", name=f"bc{t}{ih}{po}")
                        nc.gpsimd.partition_broadcast(bc[:], rs[:])
                        rc = p_nrm.tile([64, 512], F32, tag="rc", name=f"rc{t}{ih}{po}")
                        nc.vector.reciprocal_approx_fast(rc[:], bc[:])
                        if po == 0:
                            nc.vector.tensor_mul(
                                outT[t][0:64, ih * 512:(ih + 1) * 512],
                                pp[0:HD, :], rc[:])
                        else:
                            ob = p_nrm.tile([64, 512], PROJ_DT, tag="ob", name=f"ob{t}{ih}")
                            nc.vector.tensor_mul(ob[:], pp[0:HD, :], rc[:])
                            nc.sync.dma_start(
                                outT[t][64:128, ih * 512:(ih + 1) * 512], ob[:])

            for t5 in range(CB):
                nc.sync.dma_start(wp[t5][:], w_proj.ap()[t5 * 128:(t5 + 1) * 128, :])

            # ---- proj + bias + store
            for nb in range(NB):
                ys = p_y.tile([128, C], F32, tag="ys", name=f"ys{nb}")
                for cp in range(2):
                    py = ps_mm.tile([128, 384], F32, tag="pmm", name=f"py{nb}{cp}")
                    for t2 in range(CB):
                        nc.tensor.matmul(
                            py[:], outT[t2][:, nb * 128:(nb + 1) * 128],
                            wp[t2][:, cp * 384:(cp + 1) * 384],
                            start=(t2 == 0), stop=(t2 == CB - 1))
                    nc.vector.tensor_add(
                        ys[:, cp * 384:(cp + 1) * 384], py[:],
                        bias_bc[:, cp * 384:(cp + 1) * 384])
                nc.sync.dma_start(y.ap()[nb * 128:(nb + 1) * 128, :], ys[:])

    nc.compile()
    return nc


def _get_nc():
    global _NC
    if _NC is None:
        _NC = _build()
    return _NC


def _run(in_maps, trace=False, tmpdir=None):
    return bass_utils.run_bass_kernel_spmd(
        _get_nc(), in_maps, core_ids=list(range(B)), trace=trace, tmpdir=tmpdir)


def _in_maps(x, w_qkv, w_proj, b_proj):
    import ml_dtypes
    bf = ml_dtypes.bfloat16
    x = np.ascontiguousarray(np.asarray(x, dtype=np.float32).astype(bf))
    w_qkv = np.ascontiguousarray(np.asarray(w_qkv, dtype=np.float32).astype(bf))
    w_proj = np.ascontiguousarray(np.asarray(w_proj, dtype=np.float32).astype(bf))
    b_proj = np.ascontiguousarray(np.asarray(b_proj, dtype=np.float32)).reshape(1, C)
    return [
        {"x": np.ascontiguousarray(x[b]), "w_qkv": w_qkv,
         "w_proj": w_proj, "b_proj": b_proj}
        for b in range(B)
    ]


def kernel(x, w_qkv, w_proj, b_proj):
    res = _run(_in_maps(x, w_qkv, w_proj, b_proj))
    return np.stack([res.results[b]["y"] for b in range(B)], axis=0)



# revision 2
# speedup vs baseline: 1.1883x; 1.1883x over previous
"""Multi-head attention forward (B=8, N=1024, C=768, H=12) on 8 TRN2 NeuronCores.

Sharding: data-parallel over batch — core b computes batch b end-to-end
(weights replicated, no collectives). Host prep: x is pre-transposed to
xT [C, N] and cast bf16 (so no PE transposes are needed on device), the
weights are pre-cast bf16, b_proj is reshaped to [128, CB], and the
output y is produced transposed (yT [C, N]) and un-transposed on host.

Per-core dataflow, all matmuls bf16 with fp32 PSUM accumulation:

  qT,kT[t] = w_qkv-stationary matmuls over xT               [feat, seq]
  v        = xT-stationary matmuls over w_qkv[:, 1536:]     [seq, feat(+ones)]
  S^T  = kT-stationary matmuls over qT (2 heads in PE row groups, A/B
         quadrant matmuls issued adjacently so they run concurrently)
  E^T  = exp(S^T / 8) via ACT straight from PSUM (no max-sub needed)
  PV   = v_aug-stationary matmuls over E^T -> [out^T ; rowsum] in PSUM
  out^T = PV[0:64] * (1/rowsum)   (gpsimd partition-broadcast + fast recip)
  y^T = w_proj-stationary matmuls over out^T; bias added per-partition by
        the scalar engine (Identity activation with AP bias) from PSUM.

Scheduling: per (t, ih) the scores for jp+1 are emitted BEFORE PV(jp), so
the PE never waits on the exp of the tile it is about to consume; qkv/v
matmul groups are woven into fixed slots of every (t, ih) unit to fill
the PE while ACT streams exps; the ACT exp table is preloaded during the
input DMA with a dummy activation.
"""
import numpy as np
from contextlib import ExitStack

import concourse.bacc as bacc
import concourse.tile as tile
from concourse import mybir, bass_utils

F32 = mybir.dt.float32
BF16 = mybir.dt.bfloat16
EXP = mybir.ActivationFunctionType.Exp

B = 8
N = 1024       # sequence length
C = 768        # channels
H = 12         # heads
HD = 64        # head dim
NB = N // 128  # 8 seq blocks
CB = C // 128  # 6 channel chunks
HP = H // 2    # 6 head pairs
VW = HD + 1    # 65: v columns per head incl. ones column
SCALE = float(HD) ** -0.5

_NC = None


def _build():
    nc = bacc.Bacc("TRN2", target_bir_lowering=False, debug=False, num_devices=B)
    xTd = nc.dram_tensor("xT", [C, N], BF16, kind="ExternalInput")
    w_qkv = nc.dram_tensor("w_qkv", [C, 3 * C], BF16, kind="ExternalInput")
    w_proj = nc.dram_tensor("w_proj", [C, C], BF16, kind="ExternalInput")
    b_proj = nc.dram_tensor("b_proj", [128, CB], F32, kind="ExternalInput")
    y = nc.dram_tensor("y", [C, N], F32, kind="ExternalOutput")

    with tile.TileContext(nc) as tc, ExitStack() as ctx:
        const = ctx.enter_context(tc.tile_pool(name="const", bufs=1))
        p_qk = ctx.enter_context(tc.tile_pool(name="p_qk", bufs=1))
        p_v = ctx.enter_context(tc.tile_pool(name="p_v", bufs=1))
        p_out = ctx.enter_context(tc.tile_pool(name="p_out", bufs=1))
        p_wp = ctx.enter_context(tc.tile_pool(name="p_wp", bufs=1))
        p_xT = ctx.enter_context(tc.tile_pool(name="p_xT", bufs=1))
        p_wq = ctx.enter_context(tc.tile_pool(name="p_wq", bufs=1))

        bias_col = const.tile([128, CB], F32, tag="bias_col")
        ones12 = const.tile([128, H], F32, tag="ones12")
        warm_in = const.tile([1, 8], F32, tag="warm_in")
        warm_out = const.tile([1, 8], F32, tag="warm_out")

        qT = [p_qk.tile([128, N], BF16, tag=f"qT{t}", name=f"qT{t}") for t in range(HP)]
        kT = [p_qk.tile([128, N], BF16, tag=f"kT{t}", name=f"kT{t}") for t in range(HP)]
        vn = [p_v.tile([128, H * VW], BF16, tag=f"v{ib}", name=f"v{ib}") for ib in range(NB)]
        outT = [p_out.tile([128, N], BF16, tag=f"outT{t}", name=f"outT{t}") for t in range(HP)]
        wp = [p_wp.tile([128, C], BF16, tag=f"wp{t}", name=f"wp{t}") for t in range(CB)]
        xT = [p_xT.tile([128, N], BF16, tag=f"xT{c}", name=f"xT{c}") for c in range(CB)]
        wq = [p_wq.tile([128, 3 * C], BF16, tag=f"wq{c}", name=f"wq{c}") for c in range(CB)]

        with (
            tc.tile_pool(name="p_E", bufs=7) as p_E,
            tc.tile_pool(name="p_nrm", bufs=2) as p_nrm,
            tc.tile_pool(name="p_ys", bufs=2) as p_ys,
            tc.tile_pool(name="ps_mm", bufs=2, space="PSUM") as ps_mm,
            tc.tile_pool(name="ps_s", bufs=2, space="PSUM") as ps_s,
            tc.tile_pool(name="ps_pv", bufs=1, space="PSUM") as ps_pv,
        ):
            # ---- input DMAs in first-use order
            for cc in range(CB):
                nc.sync.dma_start(xT[cc][:], xTd.ap()[cc * 128:(cc + 1) * 128, :])
            # preload the exp table while DMA streams
            nc.vector.memset(warm_in[:], 0.0)
            nc.scalar.activation(warm_out[:], warm_in[:], EXP, scale=SCALE)
            nc.vector.memset(ones12[:], 1.0)
            for cc in range(CB):
                nc.sync.dma_start(wq[cc][:, C:2 * C],
                                  w_qkv.ap()[cc * 128:(cc + 1) * 128, C:2 * C])
            for cc in range(CB):
                nc.sync.dma_start(wq[cc][:, 0:C],
                                  w_qkv.ap()[cc * 128:(cc + 1) * 128, 0:C])
            for cc in range(CB):
                nc.sync.dma_start(wq[cc][:, 2 * C:3 * C],
                                  w_qkv.ap()[cc * 128:(cc + 1) * 128, 2 * C:3 * C])
            for t5 in range(CB):
                nc.sync.dma_start(wp[t5][:], w_proj.ap()[t5 * 128:(t5 + 1) * 128, :])
            nc.sync.dma_start(bias_col[:], b_proj.ap())

            def emit_qk_group(t, gi):
                # gi: 0=q nh0, 1=q nh1, 2=k nh0, 3=k nh1
                f_off, dst = ((0, qT), (C, kT))[gi // 2]
                nh = gi % 2
                pq = ps_mm.tile([128, 512], F32, tag="pmm", name=f"pq{t}_{gi}")
                for cc in range(CB):
                    nc.tensor.matmul(
                        pq[:],
                        wq[cc][:, f_off + t * 128: f_off + (t + 1) * 128],
                        xT[cc][:, nh * 512:(nh + 1) * 512],
                        start=(cc == 0), stop=(cc == CB - 1))
                nc.vector.tensor_copy(dst[t][:, nh * 512:(nh + 1) * 512], pq[:])

            def emit_v_group(ib, half):
                pv = ps_mm.tile([128, 384], F32, tag="pmm", name=f"pv{ib}_{half}")
                for cc in range(CB):
                    nc.tensor.matmul(
                        pv[:],
                        xT[cc][:, ib * 128:(ib + 1) * 128],
                        wq[cc][:, 2 * C + half * 384: 2 * C + (half + 1) * 384],
                        start=(cc == 0), stop=(cc == CB - 1))
                nc.vector.tensor_copy(
                    vn[ib][:, half * 6 * VW:(half + 1) * 6 * VW]
                    .rearrange("p (h d) -> p h d", d=VW)[:, :, 0:HD],
                    pv[:].rearrange("p (h d) -> p h d", d=HD))
                if half == 1:
                    nc.vector.tensor_copy(
                        vn[ib][:].rearrange("p (h d) -> p h d", d=VW)[:, :, HD:VW],
                        ones12[:])

            # ---- prologue: q/k for t=0 (k first), v for seq blocks 0-1
            for gi in (2, 3, 0):
                emit_qk_group(0, gi)
            for ib0 in range(2):
                for h0 in range(2):
                    emit_v_group(ib0, h0)

            # ---- weave schedule: slots (t, ih, slot) with slot 0 = pre-slot
            weave = {}
            weave[(0, 0, 0)] = [(emit_v_group, (2, 0)), (emit_v_group, (2, 1))]
            weave[(0, 0, 1)] = [(emit_v_group, (3, 0)), (emit_v_group, (3, 1))]
            weave[(0, 0, 2)] = [(emit_v_group, (4, 0)), (emit_v_group, (4, 1))]
            weave[(0, 0, 3)] = [(emit_v_group, (5, 0)), (emit_v_group, (5, 1)),
                                (emit_v_group, (6, 0)), (emit_v_group, (6, 1))]
            weave[(0, 0, 4)] = [(emit_v_group, (7, 0)), (emit_v_group, (7, 1)),
                                (emit_qk_group, (0, 1))]
            for s, gi in zip(range(4), (2, 3, 0, 1)):
                weave[(0, 1, s)] = [(emit_qk_group, (1, gi))]
            for t in range(1, HP - 1):
                weave[(t, 0, 0)] = [(emit_qk_group, (t + 1, 2))]
                weave[(t, 0, 2)] = [(emit_qk_group, (t + 1, 3))]
                weave[(t, 1, 0)] = [(emit_qk_group, (t + 1, 0))]
                weave[(t, 1, 2)] = [(emit_qk_group, (t + 1, 1))]

            sbufs = {}

            def emit_scores(t, ih, jp):
                sA = ps_s.tile([128, 1024], F32, tag="s2", name=f"sA{t}{ih}{jp}")
                sB = ps_s.tile([128, 1024], F32, tag="s2", name=f"sB{t}{ih}{jp}")
                for jb, co in zip((2 * jp, 2 * jp + 1), (0, 512)):
                    nc.tensor.matmul(
                        sA[:, co:co + 512],
                        kT[t][0:64, jb * 128:(jb + 1) * 128],
                        qT[t][0:64, ih * 512:(ih + 1) * 512],
                        start=True, stop=True, tile_position=(0, 0))
                    nc.tensor.matmul(
                        sB[:, co:co + 512],
                        kT[t][64:128, jb * 128:(jb + 1) * 128],
                        qT[t][64:128, ih * 512:(ih + 1) * 512],
                        start=True, stop=True, tile_position=(64, 0))
                sbufs[(t, ih, jp)] = (sA, sB)

            ebufs = {}

            def emit_exp(t, ih, jp):
                sA, sB = sbufs.pop((t, ih, jp))
                eA = p_E.tile([128, 1024], BF16, tag="e2", name=f"eA{t}{ih}{jp}")
                eB = p_E.tile([128, 1024], BF16, tag="e2", name=f"eB{t}{ih}{jp}")
                nc.scalar.activation(eA[:], sA[:], EXP, scale=SCALE)
                nc.scalar.activation(eB[:], sB[:], EXP, scale=SCALE)
                ebufs[(t, ih, jp)] = (eA, eB)

            def emit_pv(t, ih, jp, ppA, ppB):
                hA, hB = 2 * t, 2 * t + 1
                eA, eB = ebufs.pop((t, ih, jp))
                for pp, e, h in ((ppA, eA, hA), (ppB, eB, hB)):
                    for jb, co in zip((2 * jp, 2 * jp + 1), (0, 512)):
                        nc.tensor.matmul(
                            pp[:], vn[jb][:, h * VW:(h + 1) * VW],
                            e[:, co:co + 512],
                            start=(jb == 0), stop=(jb == NB - 1))

            # ---- attention over head pairs, queries split in halves
            for t in range(HP):
                for ih in range(2):
                    ppA = ps_pv.tile([VW, 512], F32, tag="pvA", name=f"ppA{t}{ih}")
                    ppB = ps_pv.tile([VW, 512], F32, tag="pvB", name=f"ppB{t}{ih}")
                    emit_scores(t, ih, 0)
                    emit_exp(t, ih, 0)
                    for fn, args in weave.get((t, ih, 0), ()):
                        fn(*args)
                    for jp in range(4):
                        if jp < 3:
                            emit_scores(t, ih, jp + 1)
                            emit_exp(t, ih, jp + 1)
                        for fn, args in weave.get((t, ih, jp + 1), ()):
                            fn(*args)
                        emit_pv(t, ih, jp, ppA, ppB)
                    # normalize: out^T = PV[0:64] / rowsum
                    for pp, po in ((ppA, 0), (ppB, 64)):
                        rs = p_nrm.tile([1, 512], F32, tag="rs", name=f"rs{t}{ih}{po}")
                        nc.vector.tensor_copy(rs[:], pp[HD:VW, :])
                        rc = p_nrm.tile([1, 512], F32, tag="rc", name=f"rc{t}{ih}{po}")
                        nc.vector.reciprocal_approx_fast(rc[:], rs[:])
                        bc = p_nrm.tile([64, 512], F32, tag="bc", name=f"bc{t}{ih}{po}")
                        nc.gpsimd.partition_broadcast(bc[:], rc[:])
                        if po == 0:
                            nc.vector.tensor_mul(
                                outT[t][0:64, ih * 512:(ih + 1) * 512],
                                pp[0:HD, :], bc[:])
                        else:
                            ob = p_nrm.tile([64, 512], BF16, tag="ob", name=f"ob{t}{ih}")
                            nc.vector.tensor_mul(ob[:], pp[0:HD, :], bc[:])
                            nc.sync.dma_start(
                                outT[t][64:128, ih * 512:(ih + 1) * 512], ob[:])

            # ---- proj in y^T layout + per-partition bias on ACT + store
            for cb in range(CB):
                for sh in range(2):
                    py = ps_mm.tile([128, 512], F32, tag="pmm", name=f"py{cb}{sh}")
                    for t2 in range(CB):
                        nc.tensor.matmul(
                            py[:], wp[t2][:, cb * 128:(cb + 1) * 128],
                            outT[t2][:, sh * 512:(sh + 1) * 512],
                            start=(t2 == 0), stop=(t2 == CB - 1))
                    ys = p_ys.tile([128, 512], F32, tag="ys", name=f"ys{cb}{sh}")
                    nc.scalar.add(ys[:], py[:], bias_col[:, cb:cb + 1])
                    nc.sync.dma_start(
                        y.ap()[cb * 128:(cb + 1) * 128, sh * 512:(sh + 1) * 512],
                        ys[:])

    nc.compile()
    return nc


def _get_nc():
    global _NC
    if _NC is None:
        _NC = _build()
    return _NC


def _run(in_maps, trace=False, tmpdir=None):
    return bass_utils.run_bass_kernel_spmd(
        _get_nc(), in_maps, core_ids=list(range(B)), trace=trace, tmpdir=tmpdir)


def _in_maps(x, w_qkv, w_proj, b_proj):
    import ml_dtypes
    bf = ml_dtypes.bfloat16
    x = np.asarray(x, dtype=np.float32)
    xT = np.ascontiguousarray(x.transpose(0, 2, 1).astype(bf))  # [B, C, N]
    w_qkv = np.ascontiguousarray(np.asarray(w_qkv, dtype=np.float32).astype(bf))
    w_proj = np.ascontiguousarray(np.asarray(w_proj, dtype=np.float32).astype(bf))
    b_col = np.ascontiguousarray(
        np.asarray(b_proj, dtype=np.float32).reshape(CB, 128).T)  # [128, CB]
    return [
        {"xT": np.ascontiguousarray(xT[b]), "w_qkv": w_qkv,
         "w_proj": w_proj, "b_proj": b_col}
        for b in range(B)
    ]


def kernel(x, w_qkv, w_proj, b_proj):
    res = _run(_in_maps(x, w_qkv, w_proj, b_proj))
    # device emits y^T [C, N]; un-transpose per batch
    return np.stack([res.results[b]["y"].T for b in range(B)], axis=0)


# revision 5
# speedup vs baseline: 1.2365x; 1.0406x over previous
"""Multi-head attention forward (B=8, N=1024, C=768, H=12) on 8 TRN2 NeuronCores.

Sharding: data-parallel over batch — core b computes batch b end-to-end
(weights replicated, no collectives). Host prep: x is pre-transposed to
xT [C, N] and cast bf16 (so no PE transposes are needed on device), the
weights are pre-cast bf16, b_proj is reshaped to [128, CB], and the
output y is produced transposed (yT [C, N]) and un-transposed on host.

Per-core dataflow, all matmuls bf16 with fp32 PSUM accumulation:

  qT,kT[t] = w_qkv-stationary matmuls over xT               [feat, seq]
  v        = xT-stationary matmuls over w_qkv[:, 1536:]     [seq, feat(+ones)]
  S^T  = kT-stationary matmuls over qT (2 heads in PE row groups, A/B
         quadrant matmuls issued adjacently so they run concurrently)
  E^T  = exp(S^T / 8) via ACT straight from PSUM (no max-sub needed)
  PV   = v_aug-stationary matmuls over E^T -> [out^T ; rowsum] in PSUM
  out^T = PV[0:64] * (1/rowsum)   (fast recip from PSUM + gpsimd bcast)
  y^T = w_proj-stationary matmuls over out^T; bias added per-partition by
        the scalar engine (Identity activation with AP bias) from PSUM.

Scheduling: inputs land via a few large 3D-AP DMAs ordered by first use
(x, k-cols, t0 q-slice, v-cols, rest); per (t, ih) the scores for jp+1
are emitted BEFORE PV(jp) so the PE never waits on the exp it is about
to consume; qkv/v matmul groups are woven into fixed slots of every
(t, ih) unit; the seq-half-0 projection is woven into the last (t=5,
ih=1) unit (it only needs the ih=0 halves of out^T) so only the
seq-half-1 projection remains as tail; the ACT exp table is preloaded
during the input DMA with a dummy activation.
"""
import numpy as np
from contextlib import ExitStack

import concourse.bacc as bacc
import concourse.tile as tile
from concourse import mybir, bass_utils

F32 = mybir.dt.float32
BF16 = mybir.dt.bfloat16
EXP = mybir.ActivationFunctionType.Exp

B = 8
N = 1024       # sequence length
C = 768        # channels
H = 12         # heads
HD = 64        # head dim
NB = N // 128  # 8 seq blocks
CB = C // 128  # 6 channel chunks
HP = H // 2    # 6 head pairs
VW = HD + 1    # 65: v columns per head incl. ones column
W3 = 3 * C     # per-chunk w_qkv row width
SCALE = float(HD) ** -0.5

_NC = None


def _build():
    nc = bacc.Bacc("TRN2", target_bir_lowering=False, debug=False, num_devices=B)
    xTd = nc.dram_tensor("xT", [C, N], BF16, kind="ExternalInput")
    w_qkv = nc.dram_tensor("w_qkv", [C, W3], BF16, kind="ExternalInput")
    w_proj = nc.dram_tensor("w_proj", [C, C], BF16, kind="ExternalInput")
    b_proj = nc.dram_tensor("b_proj", [128, CB], F32, kind="ExternalInput")
    y = nc.dram_tensor("y", [C, N], F32, kind="ExternalOutput")

    with tile.TileContext(nc) as tc, ExitStack() as ctx:
        const = ctx.enter_context(tc.tile_pool(name="const", bufs=1))
        p_qk = ctx.enter_context(tc.tile_pool(name="p_qk", bufs=1))
        p_v = ctx.enter_context(tc.tile_pool(name="p_v", bufs=1))
        p_out = ctx.enter_context(tc.tile_pool(name="p_out", bufs=1))
        p_wp = ctx.enter_context(tc.tile_pool(name="p_wp", bufs=1))
        p_xT = ctx.enter_context(tc.tile_pool(name="p_xT", bufs=1))
        p_wq = ctx.enter_context(tc.tile_pool(name="p_wq", bufs=1))

        bias_col = const.tile([128, CB], F32, tag="bias_col")
        ones12 = const.tile([128, H], F32, tag="ones12")
        warm_in = const.tile([1, 8], F32, tag="warm_in")
        warm_out = const.tile([1, 8], F32, tag="warm_out")

        qT = [p_qk.tile([128, N], BF16, tag=f"qT{t}", name=f"qT{t}") for t in range(HP)]
        kT = [p_qk.tile([128, N], BF16, tag=f"kT{t}", name=f"kT{t}") for t in range(HP)]
        vn = [p_v.tile([128, H * VW], BF16, tag=f"v{ib}", name=f"v{ib}") for ib in range(NB)]
        outT = [p_out.tile([128, N], BF16, tag=f"outT{t}", name=f"outT{t}") for t in range(HP)]
        wp = p_wp.tile([128, CB * C], BF16, tag="wp", name="wp")
        xT = p_xT.tile([128, CB * N], BF16, tag="xT", name="xT")
        wq = p_wq.tile([128, CB * W3], BF16, tag="wq", name="wq")

        xT3 = xT[:].rearrange("p (c n) -> p c n", n=N)
        wq3 = wq[:].rearrange("p (c s) -> p c s", s=W3)
        wp3 = wp[:].rearrange("p (c s) -> p c s", s=C)
        xTs = xTd.ap().rearrange("(c p) n -> p c n", p=128)
        wqs = w_qkv.ap().rearrange("(c p) s -> p c s", p=128)
        wps = w_proj.ap().rearrange("(c p) s -> p c s", p=128)

        with (
            tc.tile_pool(name="p_E", bufs=7) as p_E,
            tc.tile_pool(name="p_nrm", bufs=2) as p_nrm,
            tc.tile_pool(name="p_ys", bufs=2) as p_ys,
            tc.tile_pool(name="ps_mm", bufs=2, space="PSUM") as ps_mm,
            tc.tile_pool(name="ps_s", bufs=2, space="PSUM") as ps_s,
            tc.tile_pool(name="ps_pv", bufs=1, space="PSUM") as ps_pv,
        ):
            # ---- input DMAs, large 3D-AP transfers in first-use order
            nc.sync.dma_start(xT3[:, 0:3, :], xTs[:, 0:3, :])
            nc.vector.memset(warm_in[:], 0.0)
            nc.scalar.activation(warm_out[:], warm_in[:], EXP, scale=SCALE)
            nc.vector.memset(ones12[:], 1.0)
            nc.sync.dma_start(wq3[:, 0:3, C:2 * C], wqs[:, 0:3, C:2 * C])
            nc.sync.dma_start(xT3[:, 3:6, :], xTs[:, 3:6, :])
            nc.sync.dma_start(wq3[:, 3:6, C:2 * C], wqs[:, 3:6, C:2 * C])
            nc.sync.dma_start(wq3[:, :, 0:128], wqs[:, :, 0:128])  # q cols, t=0
            nc.sync.dma_start(wq3[:, 0:3, 2 * C:W3], wqs[:, 0:3, 2 * C:W3])
            nc.sync.dma_start(wq3[:, 3:6, 2 * C:W3], wqs[:, 3:6, 2 * C:W3])
            nc.sync.dma_start(wq3[:, :, 128:C], wqs[:, :, 128:C])  # q cols, t>0
            nc.sync.dma_start(wp3[:, :, :], wps[:, :, :])
            nc.sync.dma_start(bias_col[:], b_proj.ap())

            def emit_qk_group(t, gi):
                # gi: 0=q nh0, 1=q nh1, 2=k nh0, 3=k nh1
                f_off, dst = ((0, qT), (C, kT))[gi // 2]
                nh = gi % 2
                pq = ps_mm.tile([128, 512], F32, tag="pmm", name=f"pq{t}_{gi}")
                for cc in range(CB):
                    nc.tensor.matmul(
                        pq[:],
                        wq[:, cc * W3 + f_off + t * 128: cc * W3 + f_off + (t + 1) * 128],
                        xT[:, cc * N + nh * 512: cc * N + (nh + 1) * 512],
                        start=(cc == 0), stop=(cc == CB - 1))
                nc.vector.tensor_copy(dst[t][:, nh * 512:(nh + 1) * 512], pq[:])

            def emit_v_group(ib, half):
                pv = ps_mm.tile([128, 384], F32, tag="pmm", name=f"pv{ib}_{half}")
                for cc in range(CB):
                    nc.tensor.matmul(
                        pv[:],
                        xT[:, cc * N + ib * 128: cc * N + (ib + 1) * 128],
                        wq[:, cc * W3 + 2 * C + half * 384: cc * W3 + 2 * C + (half + 1) * 384],
                        start=(cc == 0), stop=(cc == CB - 1))
                nc.vector.tensor_copy(
                    vn[ib][:, half * 6 * VW:(half + 1) * 6 * VW]
                    .rearrange("p (h d) -> p h d", d=VW)[:, :, 0:HD],
                    pv[:].rearrange("p (h d) -> p h d", d=HD))
                if half == 1:
                    nc.vector.tensor_copy(
                        vn[ib][:].rearrange("p (h d) -> p h d", d=VW)[:, :, HD:VW],
                        ones12[:])

            def emit_proj(cb, sh):
                py = ps_mm.tile([128, 512], F32, tag="pmm", name=f"py{cb}{sh}")
                for t2 in range(CB):
                    nc.tensor.matmul(
                        py[:], wp[:, t2 * C + cb * 128: t2 * C + (cb + 1) * 128],
                        outT[t2][:, sh * 512:(sh + 1) * 512],
                        start=(t2 == 0), stop=(t2 == CB - 1))
                ys = p_ys.tile([128, 512], F32, tag="ys", name=f"ys{cb}{sh}")
                nc.scalar.add(ys[:], py[:], bias_col[:, cb:cb + 1])
                nc.sync.dma_start(
                    y.ap()[cb * 128:(cb + 1) * 128, sh * 512:(sh + 1) * 512],
                    ys[:])

            # ---- prologue: q/k for t=0 (k first), v for seq blocks 0-1
            for gi in (2, 3, 0):
                emit_qk_group(0, gi)
            for ib0 in range(2):
                for h0 in range(2):
                    emit_v_group(ib0, h0)

            # ---- weave schedule: slots (t, ih, slot), slot 0 = pre-slot
            weave = {}
            weave[(0, 0, 0)] = [(emit_v_group, (2, 0)), (emit_v_group, (2, 1))]
            weave[(0, 0, 1)] = [(emit_v_group, (3, 0)), (emit_v_group, (3, 1))]
            weave[(0, 0, 2)] = [(emit_v_group, (4, 0)), (emit_v_group, (4, 1))]
            weave[(0, 0, 3)] = [(emit_v_group, (5, 0)), (emit_v_group, (5, 1)),
                                (emit_v_group, (6, 0)), (emit_v_group, (6, 1))]
            weave[(0, 0, 4)] = [(emit_v_group, (7, 0)), (emit_v_group, (7, 1)),
                                (emit_qk_group, (0, 1))]
            for s, gi in zip(range(4), (2, 3, 0, 1)):
                weave[(0, 1, s)] = [(emit_qk_group, (1, gi))]
            for t in range(1, HP - 1):
                weave[(t, 0, 0)] = [(emit_qk_group, (t + 1, 2))]
                weave[(t, 0, 2)] = [(emit_qk_group, (t + 1, 3))]
                weave[(t, 1, 0)] = [(emit_qk_group, (t + 1, 0))]
                weave[(t, 1, 2)] = [(emit_qk_group, (t + 1, 1))]
            # seq-half-0 projection rides inside (5, 1): needs only ih=0
            # halves of outT, all complete after the (5, 0) normalize.
            weave[(5, 1, 1)] = [(emit_proj, (0, 0))]
            weave[(5, 1, 2)] = [(emit_proj, (1, 0)), (emit_proj, (2, 0))]
            weave[(5, 1, 3)] = [(emit_proj, (3, 0)), (emit_proj, (4, 0))]
            weave[(5, 1, 4)] = [(emit_proj, (5, 0))]

            sbufs = {}

            def emit_scores(t, ih, jp):
                sA = ps_s.tile([128, 1024], F32, tag="s2", name=f"sA{t}{ih}{jp}")
                sB = ps_s.tile([128, 1024], F32, tag="s2", name=f"sB{t}{ih}{jp}")
                for jb, co in zip((2 * jp, 2 * jp + 1), (0, 512)):
                    nc.tensor.matmul(
                        sA[:, co:co + 512],
                        kT[t][0:64, jb * 128:(jb + 1) * 128],
                        qT[t][0:64, ih * 512:(ih + 1) * 512],
                        start=True, stop=True, tile_position=(0, 0))
                    nc.tensor.matmul(
                        sB[:, co:co + 512],
                        kT[t][64:128, jb * 128:(jb + 1) * 128],
                        qT[t][64:128, ih * 512:(ih + 1) * 512],
                        start=True, stop=True, tile_position=(64, 0))
                sbufs[(t, ih, jp)] = (sA, sB)

            ebufs = {}

            def emit_exp(t, ih, jp):
                sA, sB = sbufs.pop((t, ih, jp))
                eA = p_E.tile([128, 1024], BF16, tag="e2", name=f"eA{t}{ih}{jp}")
                eB = p_E.tile([128, 1024], BF16, tag="e2", name=f"eB{t}{ih}{jp}")
                nc.scalar.activation(eA[:], sA[:], EXP, scale=SCALE)
                nc.scalar.activation(eB[:], sB[:], EXP, scale=SCALE)
                ebufs[(t, ih, jp)] = (eA, eB)

            def emit_pv(t, ih, jp, ppA, ppB):
                hA, hB = 2 * t, 2 * t + 1
                eA, eB = ebufs.pop((t, ih, jp))
                for pp, e, h in ((ppA, eA, hA), (ppB, eB, hB)):
                    for jb, co in zip((2 * jp, 2 * jp + 1), (0, 512)):
                        nc.tensor.matmul(
                            pp[:], vn[jb][:, h * VW:(h + 1) * VW],
                            e[:, co:co + 512],
                            start=(jb == 0), stop=(jb == NB - 1))

            # ---- attention over head pairs, queries split in halves
            for t in range(HP):
                for ih in range(2):
                    ppA = ps_pv.tile([VW, 512], F32, tag="pvA", name=f"ppA{t}{ih}")
                    ppB = ps_pv.tile([VW, 512], F32, tag="pvB", name=f"ppB{t}{ih}")
                    emit_scores(t, ih, 0)
                    emit_exp(t, ih, 0)
                    for fn, args in weave.get((t, ih, 0), ()):
                        fn(*args)
                    for jp in range(4):
                        if jp < 3:
                            emit_scores(t, ih, jp + 1)
                            emit_exp(t, ih, jp + 1)
                        for fn, args in weave.get((t, ih, jp + 1), ()):
                            fn(*args)
                        emit_pv(t, ih, jp, ppA, ppB)
                    # normalize: out^T = PV[0:64] / rowsum
                    for pp, po in ((ppA, 0), (ppB, 64)):
                        rs = p_nrm.tile([1, 512], F32, tag="rs", name=f"rs{t}{ih}{po}")
                        nc.vector.tensor_copy(rs[:], pp[HD:VW, :])
                        rc = p_nrm.tile([1, 512], F32, tag="rc", name=f"rc{t}{ih}{po}")
                        nc.vector.reciprocal_approx_fast(rc[:], rs[:])
                        bc = p_nrm.tile([64, 512], F32, tag="bc", name=f"bc{t}{ih}{po}")
                        nc.gpsimd.partition_broadcast(bc[:], rc[:])
                        if po == 0:
                            nc.vector.tensor_mul(
                                outT[t][0:64, ih * 512:(ih + 1) * 512],
                                pp[0:HD, :], bc[:])
                        else:
                            ob = p_nrm.tile([64, 512], BF16, tag="ob", name=f"ob{t}{ih}")
                            nc.vector.tensor_mul(ob[:], pp[0:HD, :], bc[:])
                            nc.sync.dma_start(
                                outT[t][64:128, ih * 512:(ih + 1) * 512], ob[:])

            # ---- remaining projection (seq half 1) + store
            for cb in range(CB):
                emit_proj(cb, 1)

    nc.compile()
    return nc


def _get_nc():
    global _NC
    if _NC is None:
        _NC = _build()
    return _NC


def _run(in_maps, trace=False, tmpdir=None):
    return bass_utils.run_bass_kernel_spmd(
        _get_nc(), in_maps, core_ids=list(range(B)), trace=trace, tmpdir=tmpdir)


def _in_maps(x, w_qkv, w_proj, b_proj):
    import ml_dtypes
    bf = ml_dtypes.bfloat16
    x = np.asarray(x, dtype=np.float32)
    xT = np.ascontiguousarray(x.transpose(0, 2, 1).astype(bf))  # [B, C, N]
    w_qkv = np.ascontiguousarray(np.asarray(w_qkv, dtype=np.float32).astype(bf))
    w_proj = np.ascontiguousarray(np.asarray(w_proj, dtype=np.float32).astype(bf))
    b_col = np.ascontiguousarray(
        np.asarray(b_proj, dtype=np.float32).reshape(CB, 128).T)  # [128, CB]
    return [
        {"xT": np.ascontiguousarray(xT[b]), "w_qkv": w_qkv,
         "w_proj": w_proj, "b_proj": b_col}
        for b in range(B)
    ]


def kernel(x, w_qkv, w_proj, b_proj):
    res = _run(_in_maps(x, w_qkv, w_proj, b_proj))
    # device emits y^T [C, N]; un-transpose per batch
    return np.stack([res.results[b]["y"].T for b in range(B)], axis=0)
